# revision 1
# baseline (speedup 1.0000x reference)
"""Multi-head cross-attention TRN2 kernel.

N=4096, D=256, H=4, K=16. Data-parallel over 8 NeuronCores: each core owns
512 query rows, key_value + weights replicated. No collectives.

Math (per core, rows R=512):
  QT_h [16,R]   = Wq_h.T @ q^T           (q^T via DMA transpose)
  KhT_h [16,N]  = Wk_h.T @ kv^T          (kv^T via DMA transpose)
  V_aug [N,68]  = kv @ Wv_aug            (per-head 17-col groups: 16 V cols + ones col)
  per head h, per key-chunk m (128 keys):
    S^T[m,:] (PSUM) = KhT_h[:,m].T @ QT_h   -> exp(0.25*S^T) on ACT -> bf16
    heads_psum[17,R] += V_aug[m, h-group].T @ expS^T[m,:]  (row 16... row 0 = denom)
  hn[h-rows,R] = heads_psum * bcast(1/denom)   (stacked: heads 0,1 -> hn_A rows
                 {0:17, 32:49}; heads 2,3 -> hn_B; odd heads accumulate at PSUM
                 base partition 32 so every step is partition-aligned)
  out[Rc,256] = hn_A[:,c].T @ Wo_A + hn_B[:,c].T @ Wo_B    (one fused pair of
                 matmuls per 128-query chunk contracts all 4 heads at once)

Schedule: the ACT engine (softmax exp, ~66us busy/iter) is the bottleneck and
PE busy is within ~8% of it, so the kernel runs one flat ACT-paced pipeline of
(head, key-pair) stages. All other work -- head epilogues, W_o, output DMAs,
and the NEXT call's input DMAs + Q/K/V projections (tiles double-buffered) --
is chopped into small closures pumped from a budget-paced filler queue into
the PE/DVE slack between stages. Output DMAs ride the ACT HWDGE queue so the
next call's input DMAs (SP queue) never wait behind them.

Matmul operands must sit at base partition 0/32/64 (96 = quadrant-3 bug), and
lhsT/rhs bases must match; heads are packed two per tile at bases {0,32}:
tile A holds heads 0,1; tile B holds heads 2,3 (QT, KhT, heads-psum, hn).

Everything fed to the PE is bf16 (cast on host); accumulation fp32; output fp32.
Measured end-to-end absmax-relative error vs fp32 reference: ~4e-3.
"""
from collections import deque

import numpy as np
import ml_dtypes

import concourse.bass as bass
from concourse import bacc
import concourse.mybir as mybir
import concourse.tile as tile
from concourse.bass_utils import run_bass_kernel_spmd

N, D, H, K = 4096, 256, 4, 16
NCORES = 8
R = N // NCORES          # 512 query rows per core
G = K + 1                # 17: per-head V columns + ones column
F32 = mybir.dt.float32
BF16 = mybir.dt.bfloat16
EXPF = mybir.ActivationFunctionType.Exp
BF = ml_dtypes.bfloat16

TRACE = False
LAST_RESULTS = None

NPAIR = N // 256         # 16 pairs of 128-key chunks per head
FILL_NS = 200            # per-stage filler budget: under the priority-epilogue
                         # regime, 200 beat 170 in every paired A/B trial
                         # (45-57us vs 92us) by draining prep earlier
PUMP_CAP = 260.0         # max filler debt per stage (burst clamp)
# Enqueue the next iteration's prep at stage 8 (mid-head-0) instead of 16:
# spreads filler load into head-0's slack; won every paired trial of an
# interleaved hardware A/B against stage-16 (95.2us vs 130.8us final).
PREP_STAGE = 8
# Pump epilogue/W_o closures from a priority queue ahead of bulk prep, so
# their latency-critical recip->rb chains start as early as possible: won
# every paired trial of an interleaved hardware A/B against FIFO order.
PRI_EP = True
EP_SPACE = False         # space epilogue PE ops a stage behind DVE producers
# AV trails S by 3 stages (deeper exp-input lead in the in-order PE queue):
# won an interleaved A/B on hardware 82.3us vs 98.9us against lag-2.
AV_LAG = 3
ES_BUFS = 8              # es tile rotation depth (3 in flight + margin)

# Stages whose softmax-exp runs on DVE as a Schraudolph fast-exp (one
# tensor_scalar: int16 <- round(x*128/ln2 + bias), bitcast to bf16) instead of
# the exact exp on ACT. The approximation error largely cancels through the
# softmax's own normalization: measured end-to-end metric is 6.4e-3 even with
# ALL stages offloaded (threshold 2e-2). Splitting lets the two engines share
# the ~64-tile exp load; tune DVE_FRAC8 = tiles offloaded per 8 stages.
# Offloading WHOLE stages to DVE measured SLOWER on HW despite the
# engine-balance win: the slow DVE read (TRN2 post-op DRAIN erratum, ~2x model
# cost) holds the 2-deep s_psum ping-pong and stalls the whole pipeline ~1.1us
# per offloaded stage.
DVE_FRAC8 = ()
# Instead, on alternating stages DVE takes only a 256-col STRIP of the exp
# (Schraudolph, ~520ns effective incl drain -- well inside the stage cadence,
# so the ping-pong never stalls) while ACT's tile shrinks to 768 cols
# ((768+352)/1.2 = 933ns vs 1147ns). Both write disjoint columns of one es
# tile; the AV matmuls are unchanged.
SPLIT_EVERY = 0          # stages t with t % SPLIT_EVERY == 0 get the DVE strip
SCH_A = 0.25 * 128.0 / float(np.log(2.0))          # folds the 1/sqrt(K) scale
SCH_B = 127.0 * 128.0 - 128.0 * 0.043              # exponent bias - centering

# DVE-load trims (numerically identical transforms): reciprocal straight to
# bf16 (the rb matmul consumed a bf16 copy anyway) and paired V-projection
# copies. Cut ~5us of real DVE busy (post-DRAIN-erratum) per iteration.
TRIM = False


def _build(repeats=1):
    nc = bacc.Bacc()
    q = nc.declare_dram_parameter("q", [R, D], BF16, isOutput=False)
    kv = nc.declare_dram_parameter("kv", [N, D], BF16, isOutput=False)
    # wqkv blob: [wq_pad(128) | wk_pad(128) | wv_aug(68)] = 324 cols per d-row;
    # wq/wk padded: head h at cols 64*(h//2)+32*(h%2) .. +16, zeros between.
    wqkv = nc.declare_dram_parameter("wqkv", [656, 128], BF16, isOutput=False)
    # wo blob [64, 512]: col block 256*t = heads {2t, 2t+1}; within a block,
    # head row-groups at {0:17, 32:49}, group row 0 (the ones/denominator
    # slot) and all rows outside the groups are zero.
    wo = nc.declare_dram_parameter("wo", [64, 2 * D], BF16, isOutput=False)
    out = nc.declare_dram_parameter("out", [R, D], F32, isOutput=True)

    with tile.TileContext(nc) as tc:
        with (
            tc.tile_pool(name="consts", bufs=2) as consts,
            tc.tile_pool(name="es", bufs=ES_BUFS) as espool,
            tc.tile_pool(name="sbops", bufs=3) as sbops,
            tc.tile_pool(name="spsum", bufs=2, space="PSUM") as spsum,
            tc.tile_pool(name="hpsum", bufs=2, space="PSUM") as hpsum,
            tc.tile_pool(name="mpsum", bufs=2, space="PSUM") as mpsum,
        ):
            pipe = _Pipeline(nc, consts, espool, sbops, spsum, hpsum, mpsum,
                             q, kv, wqkv, wo, out)
            ctx = None
            for rep in range(repeats):
                if ctx is None:
                    ctx = pipe.emit_prep_inline()
                ctx_next = pipe.run_iteration(ctx, prep_next=(rep < repeats - 1))
                ctx = ctx_next
            pipe.drain()

    nc.finalize()
    return nc


class _Pipeline:
    def __init__(self, nc, consts, espool, sbops, spsum, hpsum, mpsum,
                 q, kv, wqkv, wo, out):
        self.nc = nc
        self.consts = consts
        self.espool = espool
        self.sbops = sbops
        self.spsum = spsum
        self.hpsum = hpsum
        self.mpsum = mpsum
        self.q, self.kv, self.wqkv, self.wo, self.out = q, kv, wqkv, wo, out
        self.filler = deque()       # (pe_cost_ns, closure) — bulk prep work
        self.pri = deque()          # latency-critical (epilogues, W_o/out)
        self.spent = 0.0
        self.allowance = 0.0

    def pump(self, budget_ns):
        # clamp: at most ~one matmul's worth of filler debt per stage, so a
        # backlog never bursts into the PE queue and starves the ACT engine
        self.allowance = min(self.allowance + budget_ns, self.spent + PUMP_CAP)
        while (self.pri or self.filler) and self.spent <= self.allowance:
            q = self.pri if PRI_EP and self.pri else self.filler
            cost, fn = q.popleft()
            fn()
            self.spent += cost

    def drain(self):
        while self.pri or self.filler:
            q = self.pri if self.pri else self.filler
            cost, fn = q.popleft()
            fn()

    # ---------- per-iteration resource prep (DMAs + projections) ----------

    def prep_closures(self):
        """Closures that allocate + fill one iteration's input tiles.

        Executed inline for the first iteration, or pumped into the previous
        iteration's stage slack (all tiles are double-buffered via bufs=2 on
        the consts pool)."""
        nc = self.nc
        consts = self.consts
        ctx = {}
        cls = []

        def dmas():
            wqkv_sb = consts.tile([128, 656], BF16, tag="wqkv_sb", name="wqkv_sb")
            nc.sync.dma_start(out=wqkv_sb, in_=self.wqkv[:, :], transpose=True)
            qt0 = consts.tile([128, R], BF16, tag="qt0", name="qt0")
            qt1 = consts.tile([128, R], BF16, tag="qt1", name="qt1")
            kt0 = consts.tile([128, N], BF16, tag="kt0", name="kt0")
            kt1 = consts.tile([128, N], BF16, tag="kt1", name="kt1")
            nc.sync.dma_start(out=qt0, in_=self.q[:, 0:128], transpose=True)
            nc.sync.dma_start(out=qt1, in_=self.q[:, 128:256], transpose=True)
            # 512-row chunks measure fastest (2048-row transposes were +14us)
            for j in range(N // 512):
                sl = slice(512 * j, 512 * (j + 1))
                nc.sync.dma_start(out=kt0[:, sl], in_=self.kv[sl, 0:128],
                                  transpose=True)
                nc.sync.dma_start(out=kt1[:, sl], in_=self.kv[sl, 128:256],
                                  transpose=True)
            wo_sb = consts.tile([64, 2 * D], BF16, tag="wo_sb", name="wo_sb")
            nc.sync.dma_start(out=wo_sb, in_=self.wo[:, :])
            ones = consts.tile([33, G], BF16, tag="ones", name="ones")
            nc.vector.memset(ones, 1.0)
            ctx.update(wqkv_sb=wqkv_sb, qt0=qt0, qt1=qt1, kt0=kt0, kt1=kt1,
                       wo_sb=wo_sb, ones=ones)

        def qt_proj():
            wqkv_sb, qt0, qt1 = ctx["wqkv_sb"], ctx["qt0"], ctx["qt1"]
            qt_sb = [consts.tile([64, R], BF16, tag=f"qt_sb{t}", name=f"qt_sb{t}")
                     for t in range(2)]
            qt_psum = self.mpsum.tile([128, R], F32, tag="m", name="m")
            nc.tensor.matmul(qt_psum[:], wqkv_sb[:, 0:128], qt0[:],
                             start=True, stop=False)
            nc.tensor.matmul(qt_psum[:], wqkv_sb[:, 324:452], qt1[:],
                             start=False, stop=True)
            nc.vector.tensor_copy(qt_sb[0][:], qt_psum[0:64, :])
            nc.vector.tensor_copy(qt_sb[1][:], qt_psum[64:128, :])
            # hn tiles must start zeroed: junk rows feed the fused W_o matmul
            # (their wo rows are zero, but NaN garbage would still poison it)
            hn = [consts.tile([64, R], BF16, tag=f"hn{t}", name=f"hn{t}")
                  for t in range(2)]
            nc.vector.memset(hn[0], 0.0)
            nc.vector.memset(hn[1], 0.0)
            kht = [consts.tile([64, N], BF16, tag=f"kht{t}", name=f"kht{t}")
                   for t in range(2)]
            v_aug = consts.tile([128, 32 * H * G], BF16, tag="v_aug", name="v_aug")
            v_ones = v_aug[:].rearrange("p (i g s) -> p i g s",
                                        g=H, s=G)[:, :, :, 0:1]
            nc.vector.memset(v_ones, 1.0)
            ctx.update(qt_sb=qt_sb, hn=hn, kht=kht, v_aug=v_aug)

        def kh_a(j):
            def f():
                kh_psum = self.mpsum.tile([128, 512], F32, tag="m", name="m")
                nc.tensor.matmul(kh_psum[:], ctx["wqkv_sb"][:, 128:256],
                                 ctx["kt0"][:, 512 * j:512 * (j + 1)],
                                 start=True, stop=False)
                ctx["kh_psum"] = kh_psum
            return f

        def kh_b(j):
            def f():
                kh_psum, kht = ctx.pop("kh_psum"), ctx["kht"]
                nc.tensor.matmul(kh_psum[:], ctx["wqkv_sb"][:, 452:580],
                                 ctx["kt1"][:, 512 * j:512 * (j + 1)],
                                 start=False, stop=True)
                nc.vector.tensor_copy(kht[0][:, 512 * j:512 * (j + 1)],
                                      kh_psum[0:64, :])
                nc.vector.tensor_copy(kht[1][:, 512 * j:512 * (j + 1)],
                                      kh_psum[64:128, :])
            return f

        def v_mm(i, half):
            def f():
                wqkv_sb, v_aug = ctx["wqkv_sb"], ctx["v_aug"]
                if not TRIM:
                    v_psum = self.mpsum.tile([128, H * G], F32, tag="m", name="m")
                    nc.tensor.matmul(v_psum[:],
                                     ctx["kt0"][:, 128 * i:128 * (i + 1)],
                                     wqkv_sb[:, 256:324], start=True, stop=False)
                    nc.tensor.matmul(v_psum[:],
                                     ctx["kt1"][:, 128 * i:128 * (i + 1)],
                                     wqkv_sb[:, 580:648], start=False, stop=True)
                    vsrc = v_psum[:].rearrange("p (g s) -> p g s",
                                               s=G)[:, :, 1:G]
                    vdst = v_aug[:, 68 * i:68 * (i + 1)].rearrange(
                        "p (g s) -> p g s", s=G)[:, :, 1:G]
                    nc.vector.tensor_copy(vdst, vsrc)
                    return
                # TRIM: two key-chunks share one [128, 136] psum tile; ONE
                # strided copy into v_aug (halves per-op DVE init + drain)
                if half == 0:
                    ctx["v_psum"] = self.mpsum.tile([128, 2 * H * G], F32,
                                                    tag="m", name="m")
                v_psum = ctx["v_psum"]
                vp = v_psum[:, 68 * half:68 * (half + 1)]
                nc.tensor.matmul(vp, ctx["kt0"][:, 128 * i:128 * (i + 1)],
                                 wqkv_sb[:, 256:324], start=True, stop=False)
                nc.tensor.matmul(vp, ctx["kt1"][:, 128 * i:128 * (i + 1)],
                                 wqkv_sb[:, 580:648], start=False, stop=True)
                if half == 1:
                    del ctx["v_psum"]
                    i0 = i - 1
                    vsrc = v_psum[:].rearrange("p (c g s) -> p c g s",
                                               c=2, s=G)[:, :, :, 1:G]
                    vdst = v_aug[:, 68 * i0:68 * (i0 + 2)].rearrange(
                        "p (c g s) -> p c g s", c=2, s=G)[:, :, :, 1:G]
                    nc.vector.tensor_copy(vdst, vsrc)
            return f

        cls.append((60.0, dmas))
        cls.append((500.0, qt_proj))
        for j in range(N // 512):
            cls.append((220.0, kh_a(j)))
            cls.append((220.0, kh_b(j)))
            for i in range(4 * j, 4 * j + 4):
                cls.append((75.0, v_mm(i, i % 2)))
        return ctx, cls

    def emit_prep_inline(self):
        ctx, cls = self.prep_closures()
        for _, fn in cls:
            fn()
        return ctx

    # ---------- one attention iteration ----------

    def run_iteration(self, ctx, prep_next):
        nc = self.nc
        es_tiles = {}
        pair_psums = {}

        def s_stage(h, p, stage_t):
            t, b = h // 2, 32 * (h % 2)
            kht_t, qt_t = ctx["kht"][t], ctx["qt_sb"][t]
            s_psum = self.spsum.tile([128, 1024], F32, tag="s", name="s")
            lo, hi = 256 * p, 256 * p + 128
            nc.tensor.matmul(s_psum[:, 0:512],
                             kht_t[b:b + 16, lo:lo + 128],
                             qt_t[b:b + 16, :], start=True, stop=True)
            nc.tensor.matmul(s_psum[:, 512:1024],
                             kht_t[b:b + 16, hi:hi + 128],
                             qt_t[b:b + 16, :], start=True, stop=True)
            if stage_t % 8 in DVE_FRAC8:
                esi = self.espool.tile([128, 1024], mybir.dt.int16,
                                       tag="esi", name="esi")
                nc.vector.tensor_scalar(esi[:], s_psum[:], SCH_A, SCH_B,
                                        op0=mybir.AluOpType.mult,
                                        op1=mybir.AluOpType.add)
                es_tiles[(h, p)] = esi.bitcast(BF16)
            elif SPLIT_EVERY and stage_t % SPLIT_EVERY == 0:
                es = self.espool.tile([128, 1024], BF16, tag="es", name="es")
                nc.scalar.activation(es[:, 0:768], s_psum[:, 0:768],
                                     EXPF, scale=0.25)
                nc.vector.tensor_scalar(es[:, 768:1024].bitcast(mybir.dt.int16),
                                        s_psum[:, 768:1024], SCH_A, SCH_B,
                                        op0=mybir.AluOpType.mult,
                                        op1=mybir.AluOpType.add)
                es_tiles[(h, p)] = es
            else:
                es = self.espool.tile([128, 1024], BF16, tag="es", name="es")
                nc.scalar.activation(es[:], s_psum[:], EXPF, scale=0.25)
                es_tiles[(h, p)] = es

        def av_stage(h, p):
            # odd heads accumulate at PSUM base partition 32 of the shared
            # pair tile so the whole epilogue stays partition-aligned
            if h % 2 == 0 and p == 0:
                pair_psums[h // 2] = self.hpsum.tile([49, R], F32, tag="heads",
                                                     name="heads")
            hp = pair_psums[h // 2]
            hb = 32 * (h % 2)
            es = es_tiles.pop((h, p))
            c0, c1 = 2 * p, 2 * p + 1
            v_aug = ctx["v_aug"]
            nc.tensor.matmul(hp[hb:hb + 17, :],
                             v_aug[:, 68 * c0 + 17 * h:68 * c0 + 17 * h + 17],
                             es[:, 0:512], start=(p == 0), stop=False)
            nc.tensor.matmul(hp[hb:hb + 17, :],
                             v_aug[:, 68 * c1 + 17 * h:68 * c1 + 17 * h + 17],
                             es[:, 512:1024], start=False, stop=(p == NPAIR - 1))

        def head_epilogue(h):
            hp = pair_psums[h // 2]
            hb = 32 * (h % 2)
            st = {}

            def ep_a():
                recipb = self.sbops.tile([33, R], BF16, tag="recipb", name="recipb")
                if TRIM:
                    with nc.allow_low_precision(
                            reason="rb matmul takes bf16 anyway; identical "
                                   "to the fp32 recip -> bf16 copy"):
                        nc.vector.reciprocal(recipb[hb:hb + 1, :],
                                             hp[hb:hb + 1, :])
                else:
                    recip = self.sbops.tile([33, R], F32, tag="recip",
                                            name="recip")
                    nc.vector.reciprocal(recip[hb:hb + 1, :], hp[hb:hb + 1, :])
                    nc.vector.tensor_copy(recipb[hb:hb + 1, :],
                                          recip[hb:hb + 1, :])
                st["recipb"] = recipb

            def ep_b():
                rb_psum = self.mpsum.tile([128, R], F32, tag="m", name="m")
                nc.tensor.matmul(rb_psum[hb:hb + 17, :],
                                 ctx["ones"][hb:hb + 1, :],
                                 st["recipb"][hb:hb + 1, :],
                                 start=True, stop=True)
                rb_sb = self.sbops.tile([49, R], F32, tag="rb_sb", name="rb_sb")
                nc.vector.tensor_copy(rb_sb[hb:hb + 17, :], rb_psum[hb:hb + 17, :])
                st["rb"] = rb_sb

            def ep_hn():
                hn_t = ctx["hn"][h // 2]
                nc.vector.tensor_mul(hn_t[hb:hb + 17, :], hp[hb:hb + 17, :],
                                     st["rb"][hb:hb + 17, :])

            # EP_SPACE: inflate the DVE-side closures' pacing cost so each
            # dependent PE op (rb, W_o) lands a full stage after its DVE
            # producer and never stalls the in-order PE queue
            ca = 260.0 if EP_SPACE else 60.0
            ch = 200.0 if EP_SPACE else 60.0
            return [(ca, ep_a), (260.0, ep_b), (ch, ep_hn)]

        def o_chunk(c):
            def f():
                cs = slice(128 * c, 128 * (c + 1))
                hn, wo_sb = ctx["hn"], ctx["wo_sb"]
                o_psum = self.mpsum.tile([128, D], F32, tag="m", name="m")
                nc.tensor.matmul(o_psum[:], hn[0][:, cs], wo_sb[:, 0:D],
                                 start=True, stop=False)
                nc.tensor.matmul(o_psum[:], hn[1][:, cs], wo_sb[:, D:2 * D],
                                 start=False, stop=True)
                o_sb = self.sbops.tile([128, D], F32, tag="o_sb", name="o_sb",
                                       bufs=2)
                nc.vector.tensor_copy(o_sb[:], o_psum[:])
                # ACT-queue HWDGE: keeps the store off the SP queue so the
                # next iteration's input DMAs aren't stuck behind it
                # (measured faster than SP-queue stores)
                nc.scalar.dma_start(out=self.out[cs, :], in_=o_sb[:])
            return f

        ctx_next, prep = (self.prep_closures() if prep_next else (None, []))
        prep = deque(prep)

        # AV trails S by AV_LAG stages: exp(t)'s inputs sit that far ahead in
        # the in-order PE queue, so filler bursts and semaphore latency can't
        # starve the ACT engine
        stages = [(h, p) for h in range(H) for p in range(NPAIR)]

        def retire(t):
            hp_prev = stages[t]
            av_stage(*hp_prev)
            if hp_prev[1] == NPAIR - 1:
                epq = self.pri if PRI_EP else self.filler
                epq.extend(head_epilogue(hp_prev[0]))
                if hp_prev[0] == H - 1:
                    for c in range(R // 128):
                        epq.append((280.0, o_chunk(c)))

        for t, (h, p) in enumerate(stages):
            s_stage(h, p, t)
            # enqueue next iteration's prep behind this iteration's first
            # epilogue so it lands in the heads 1..3 slack
            if prep and t == PREP_STAGE:
                self.filler.extend(prep)
                prep.clear()
            self.pump(FILL_NS)
            if t >= AV_LAG:
                retire(t - AV_LAG)
        for t in range(len(stages) - AV_LAG, len(stages)):
            retire(t)
        if prep:
            self.filler.extend(prep)
        return ctx_next


_NC_CACHE = None


def _host_in_maps(query, key_value, W_q, W_k, W_v, W_o):
    q_bf = np.ascontiguousarray(query.astype(BF))
    kv_bf = np.ascontiguousarray(key_value.astype(BF))
    # padded wq/wk: head h at cols 64*(h//2)+32*(h%2) .. +16
    wqkv_h = np.zeros((D, 324), dtype=BF)
    wqt = np.transpose(W_q, (1, 0, 2))  # [D, H, K]
    wkt = np.transpose(W_k, (1, 0, 2))
    wvt = np.transpose(W_v, (1, 0, 2))
    for h in range(H):
        c0 = 64 * (h // 2) + 32 * (h % 2)
        wqkv_h[:, c0:c0 + K] = wqt[:, h, :].astype(BF)
        wqkv_h[:, 128 + c0:128 + c0 + K] = wkt[:, h, :].astype(BF)
        wqkv_h[:, 256 + G * h + 1:256 + G * (h + 1)] = wvt[:, h, :].astype(BF)
    wqkv_h = np.ascontiguousarray(np.concatenate(
        [wqkv_h[0:128].T, wqkv_h[128:256].T, np.zeros((8, 128), dtype=BF)], axis=0))
    # wo blob [64, 512]: col block 256*t = heads {2t, 2t+1}; head row-groups at
    # {0:17, 32:49} with group row 0 (denominator slot) zero
    wo_h = np.zeros((64, 2 * D), dtype=BF)
    wo_r = W_o.reshape(H, K, D)
    for h in range(H):
        t, hb = h // 2, 32 * (h % 2)
        wo_h[hb + 1:hb + G, D * t:D * (t + 1)] = wo_r[h].astype(BF)
    return [{"q": q_bf[c * R:(c + 1) * R], "kv": kv_bf, "wqkv": wqkv_h, "wo": wo_h}
            for c in range(NCORES)]


def kernel(query, key_value, W_q, W_k, W_v, W_o):
    global _NC_CACHE, LAST_RESULTS
    if _NC_CACHE is None:
        _NC_CACHE = _build()
    nc = _NC_CACHE
    in_maps = _host_in_maps(query, key_value, W_q, W_k, W_v, W_o)
    res = run_bass_kernel_spmd(nc, in_maps, list(range(NCORES)), trace=TRACE)
    LAST_RESULTS = res
    return np.concatenate([res.results[c]["out"] for c in range(NCORES)], axis=0)



# revision 3
# speedup vs baseline: 7.1404x; 7.1404x over previous
"""Multi-head cross-attention TRN2 kernel, v3.

N=4096, D=256, H=4, K=16. Data-parallel over 8 NeuronCores: each core owns
R=512 query rows, key_value + weights replicated. No collectives.

Differences vs the v2 baseline (kernel.py):
  * Stage = (head-pair, 128-key chunk) instead of (head, 256-key pair).
    The two S matmuls of a stage use DIFFERENT PE row groups (heads at
    32-aligned partition bases), and the two AV matmuls use DIFFERENT col
    groups, so each pair runs CONCURRENTLY in the 128x128 array (row/col
    tile_position packing; K=16 and M=32 use 1/4 of the array each).
    PE busy drops ~61us -> ~34us.
  * Heads live at partition base 32h in ONE qt/kht tile (Q3=True) or two
    pair tiles at {0,32} (Q3=False fallback if quadrant 3 is buggy).
  * v_aug groups padded to 32 cols/head (ones slot + 16 V cols + 15 zeros)
    so AV writes full 32-row groups; the whole epilogue is then batched:
    ONE reciprocal + ONE rb-broadcast group + ONE copy + ONE hn multiply
    covers all 4 heads (DVE cost is free-dim-only). DVE epilogue ~10.5us
    -> ~2us.
  * Every stage offloads the last STRIP_X columns of the softmax exp to
    DVE as a Schraudolph fast-exp (int16 bitcast to bf16); ACT computes
    the remaining 1024-STRIP_X exactly. ACT busy 73us -> ~60us @ X=256.

Schedule: same ACT-paced flat pipeline as v2 (filler/pri queues pumped
into the PE/DVE slack between stages; next call's prep double-buffered).

Everything fed to the PE is bf16 (cast on host); accumulation fp32;
output fp32.
"""
from collections import deque

import numpy as np
import ml_dtypes

import concourse.bass as bass
from concourse import bacc
import concourse.mybir as mybir
import concourse.tile as tile
from concourse.bass_utils import run_bass_kernel_spmd

N, D, H, K = 4096, 256, 4, 16
NCORES = 8
R = N // NCORES          # 512 query rows per core
NCHUNK = N // 128        # 32 key chunks
F32 = mybir.dt.float32
BF16 = mybir.dt.bfloat16
I16 = mybir.dt.int16
EXPF = mybir.ActivationFunctionType.Exp
BF = ml_dtypes.bfloat16

TRACE = False
LAST_RESULTS = None

# --- tunables ---
Q3 = True                # heads at bases {0,32,64,96} in single tiles; if the
                         # quadrant-3 HW bug bites, flip to pair tiles {0,32}
STRIP_X = 0              # exp columns per stage offloaded to DVE (Schraudolph
                         # strip) — superseded by DVE_EVERY (the strip write
                         # false-WAWs against ACT's exp in the same tile)
SKIP_CONST_MEMSETS = True  # v_aug pads/ones are identical every iteration and
                         # the consts pool double-buffers tag-stably, so after
                         # the first two emissions the memsets are redundant
O_PAIR = True            # two W_o query-chunks share one PSUM tile + one copy
ABLATE = ""              # timing-only ablations (output garbage): comma-set of
                         # noact,nostrip,nos,noav,noprep,noepi
SPSUM3 = True            # s_psum 3-deep rotation (6 banks) + single-buffered
                         # hp/mpsum; deeper pipeline tolerance vs prep/epilogue
                         # serialization
DVE_EVERY = 4            # every k-th stage's exp runs ENTIRELY on DVE
                         # (Schraudolph into its own int16 tile -> no
                         # mixed-engine es tile -> no false WAW edge);
                         # use with STRIP_X=0
MPSUM256 = False         # mpsum tiles [128,256] x2 bufs in ONE bank (prep
                         # matmuls split 256-wide): with SPSUM3 this restores
                         # double-buffered prep so PE never head-blocks on a
                         # queued DVE copy
O_ON_SP = True           # out-store DMAs ride the SP queue instead of ACT
                         # (ACT-queue enqueues wait o_sb copies and block exp)
EPI_HPSB = False         # epilogue: copy hp->SBUF in parallel with recip/rb,
                         # then hn = hp_sb * rb_psum(PSUM); one fewer
                         # cross-engine hop on the iteration-boundary chain
FILL_NS = 200
PUMP_CAP = 260.0
PREP_STAGE = 8
AV_LAG = 3
ES_BUFS = 8

SCH_A = 0.25 * 128.0 / float(np.log(2.0))
SCH_B = 127.0 * 128.0 - 128.0 * 0.043


def _build(repeats=1):
    nc = bacc.Bacc()
    q = nc.declare_dram_parameter("q", [R, D], BF16, isOutput=False)
    kv = nc.declare_dram_parameter("kv", [N, D], BF16, isOutput=False)
    # wqkv blob [768, 128]: rows 384t..384t+384 = d-half t, cols:
    #   0:128 wq_pad | 128:256 wk_pad | 256:384 wv_pad  (transposed on DMA)
    # wq/wk_pad: head h at cols 32h..32h+16.  wv_pad: head h V at cols
    # 32h+1..32h+17 (col 32h is the ones slot, filled on device).
    wqkv = nc.declare_dram_parameter("wqkv", [768, 128], BF16, isOutput=False)
    # wo blob [128, 256]: rows 32h+1+k = W_o[h*16+k]; rows 32h and
    # 32h+17..32h+32 zero.
    wo = nc.declare_dram_parameter("wo", [128, D], BF16, isOutput=False)
    out = nc.declare_dram_parameter("out", [R, D], F32, isOutput=True)

    with tile.TileContext(nc) as tc:
        with (
            tc.tile_pool(name="persist", bufs=1) as persist,
            tc.tile_pool(name="consts", bufs=2) as consts,
            tc.tile_pool(name="es", bufs=ES_BUFS) as espool,
            tc.tile_pool(name="sbops", bufs=3) as sbops,
            tc.tile_pool(name="spsum", bufs=3 if SPSUM3 else 2,
                         space="PSUM") as spsum,
            tc.tile_pool(name="hpsum", bufs=1 if SPSUM3 else 2,
                         space="PSUM") as hpsum,
            tc.tile_pool(name="mpsum", bufs=1 if SPSUM3 else 2,
                         space="PSUM") as mpsum,
        ):
            # ones_rb const (input-independent): row 32h cols 0:17 = 1.
            ones_rb = persist.tile([128, 32], BF16, name="ones_rb")
            nc.vector.memset(ones_rb, 0.0)
            for h in range(H):
                nc.vector.memset(ones_rb[32 * h:32 * h + 1, 0:17], 1.0)

            pipe = _Pipeline(nc, consts, espool, sbops, spsum, hpsum, mpsum,
                             q, kv, wqkv, wo, out, ones_rb)
            ctx = None
            for rep in range(repeats):
                if ctx is None:
                    ctx = pipe.emit_prep_inline()
                ctx_next = pipe.run_iteration(ctx, prep_next=(rep < repeats - 1))
                ctx = ctx_next
            pipe.drain()

    nc.finalize()
    return nc


class _Pipeline:
    def __init__(self, nc, consts, espool, sbops, spsum, hpsum, mpsum,
                 q, kv, wqkv, wo, out, ones_rb):
        self.nc = nc
        self.consts = consts
        self.espool = espool
        self.sbops = sbops
        self.spsum = spsum
        self.hpsum = hpsum
        self.mpsum = mpsum
        self.q, self.kv, self.wqkv, self.wo, self.out = q, kv, wqkv, wo, out
        self.ones_rb = ones_rb
        self.filler = deque()       # (pe_cost_ns, closure) — bulk prep work
        self.pri = deque()          # latency-critical (epilogue, W_o/out)
        self.spent = 0.0
        self.allowance = 0.0
        self.prep_count = 0

    def pump(self, budget_ns):
        self.allowance = min(self.allowance + budget_ns, self.spent + PUMP_CAP)
        while (self.pri or self.filler) and self.spent <= self.allowance:
            q = self.pri if self.pri else self.filler
            cost, fn = q.popleft()
            fn()
            self.spent += cost

    def drain(self):
        while self.pri or self.filler:
            q = self.pri if self.pri else self.filler
            cost, fn = q.popleft()
            fn()

    # ---------- per-iteration resource prep (DMAs + projections) ----------

    def prep_closures(self):
        nc = self.nc
        consts = self.consts
        ctx = {}
        cls = []

        def dmas():
            wqkv_sb = consts.tile([128, 768], BF16, tag="wqkv_sb", name="wqkv_sb")
            nc.sync.dma_start(out=wqkv_sb, in_=self.wqkv[:, :], transpose=True)
            qt0 = consts.tile([128, R], BF16, tag="qt0", name="qt0")
            qt1 = consts.tile([128, R], BF16, tag="qt1", name="qt1")
            kt0 = consts.tile([128, N], BF16, tag="kt0", name="kt0")
            kt1 = consts.tile([128, N], BF16, tag="kt1", name="kt1")
            nc.sync.dma_start(out=qt0, in_=self.q[:, 0:128], transpose=True)
            nc.sync.dma_start(out=qt1, in_=self.q[:, 128:256], transpose=True)
            for j in range(N // 512):
                sl = slice(512 * j, 512 * (j + 1))
                nc.sync.dma_start(out=kt0[:, sl], in_=self.kv[sl, 0:128],
                                  transpose=True)
                nc.sync.dma_start(out=kt1[:, sl], in_=self.kv[sl, 128:256],
                                  transpose=True)
            wo_sb = consts.tile([128, D], BF16, tag="wo_sb", name="wo_sb")
            nc.sync.dma_start(out=wo_sb, in_=self.wo[:, :])
            ctx.update(wqkv_sb=wqkv_sb, qt0=qt0, qt1=qt1, kt0=kt0, kt1=kt1,
                       wo_sb=wo_sb)

        fresh = (not SKIP_CONST_MEMSETS) or self.prep_count < 2
        self.prep_count += 1

        def v_zero():
            # pads must be zero: AV reads full 32-col groups. The pads and
            # ones column are identical every iteration, and the 2-buf pool
            # rotates tag-stably, so only the first two emissions memset.
            v_aug = consts.tile([128, 32 * NCHUNK * H], BF16, tag="v_aug",
                                name="v_aug")
            if fresh:
                nc.vector.memset(v_aug, 0.0)
                ones_v = v_aug[:].rearrange("p (c h s) -> p c h s",
                                            c=NCHUNK, s=32)[:, :, :, 0:1]
                nc.vector.memset(ones_v, 1.0)
            ctx.update(v_aug=v_aug)

        MW = 256 if MPSUM256 else 512

        def qt_proj():
            wqkv_sb, qt0, qt1 = ctx["wqkv_sb"], ctx["qt0"], ctx["qt1"]
            qt = consts.tile([128, R], BF16, tag="qt", name="qt")
            for w in range(0, R, MW):
                qt_psum = self.mpsum.tile([128, MW], F32, tag="m", name="m")
                nc.tensor.matmul(qt_psum[:], wqkv_sb[:, 0:128],
                                 qt0[:, w:w + MW], start=True, stop=False)
                nc.tensor.matmul(qt_psum[:], wqkv_sb[:, 384:512],
                                 qt1[:, w:w + MW], start=False, stop=True)
                nc.vector.tensor_copy(qt[:, w:w + MW], qt_psum[:])
            kht = consts.tile([128, N], BF16, tag="kht", name="kht")
            ctx.update(qt=qt, kht=kht)

        def kh_a(j, w):
            def f():
                kh_psum = self.mpsum.tile([128, MW], F32, tag="m", name="m")
                nc.tensor.matmul(kh_psum[:], ctx["wqkv_sb"][:, 128:256],
                                 ctx["kt0"][:, w:w + MW],
                                 start=True, stop=False)
                ctx["kh_psum"] = kh_psum
            return f

        def kh_b(j, w):
            def f():
                kh_psum = ctx.pop("kh_psum")
                nc.tensor.matmul(kh_psum[:], ctx["wqkv_sb"][:, 512:640],
                                 ctx["kt1"][:, w:w + MW],
                                 start=False, stop=True)
                nc.vector.tensor_copy(ctx["kht"][:, w:w + MW], kh_psum[:])
            return f

        def v_mm(i, half):
            # chunk c = 2*i' + half; two chunks share one [128, 256] psum and
            # ONE strided copy of their V columns into v_aug
            def f():
                wqkv_sb, v_aug = ctx["wqkv_sb"], ctx["v_aug"]
                if half == 0:
                    ctx["v_psum"] = self.mpsum.tile([128, 256], F32,
                                                    tag="m", name="m")
                v_psum = ctx["v_psum"]
                vp = v_psum[:, 128 * half:128 * (half + 1)]
                nc.tensor.matmul(vp, ctx["kt0"][:, 128 * i:128 * (i + 1)],
                                 wqkv_sb[:, 256:384], start=True, stop=False)
                nc.tensor.matmul(vp, ctx["kt1"][:, 128 * i:128 * (i + 1)],
                                 wqkv_sb[:, 640:768], start=False, stop=True)
                if half == 1:
                    del ctx["v_psum"]
                    i0 = i - 1
                    vsrc = v_psum[:].rearrange("p (c h s) -> p c h s",
                                               c=2, s=32)[:, :, :, 1:17]
                    vdst = v_aug[:, 128 * i0:128 * (i0 + 2)].rearrange(
                        "p (c h s) -> p c h s", c=2, s=32)[:, :, :, 1:17]
                    nc.vector.tensor_copy(vdst, vsrc)
            return f

        cls.append((60.0, dmas))
        cls.append((60.0, v_zero))
        cls.append((500.0, qt_proj))
        kcost = 220.0 * MW / 512
        for j in range(N // 512):
            for w in range(512 * j, 512 * (j + 1), MW):
                cls.append((kcost, kh_a(j, w)))
                cls.append((kcost, kh_b(j, w)))
            for i in range(4 * j, 4 * j + 4):
                cls.append((75.0, v_mm(i, i % 2)))
        return ctx, cls

    def emit_prep_inline(self):
        ctx, cls = self.prep_closures()
        for _, fn in cls:
            fn()
        return ctx

    # ---------- one attention iteration ----------

    def run_iteration(self, ctx, prep_next):
        nc = self.nc
        es_tiles = {}
        hp_box = {}

        abl = set(ABLATE.split(",")) if ABLATE else set()

        def s_stage(P, c):
            kht, qt = ctx["kht"], ctx["qt"]
            b0 = 64 * P
            s_psum = self.spsum.tile([128, 1024], F32, tag="s", name="s")
            ck = slice(128 * c, 128 * (c + 1))
            if "nos" not in abl:
                nc.tensor.matmul(s_psum[:, 0:512],
                                 kht[b0:b0 + 16, ck], qt[b0:b0 + 16, :],
                                 start=True, stop=True, tile_position=(b0, 0))
                nc.tensor.matmul(s_psum[:, 512:1024],
                                 kht[b0 + 32:b0 + 48, ck],
                                 qt[b0 + 32:b0 + 48, :],
                                 start=True, stop=True,
                                 tile_position=(b0 + 32, 0))
            t = 32 * P + c
            if DVE_EVERY and t % DVE_EVERY == DVE_EVERY - 1:
                # whole-stage Schraudolph exp on DVE, own int16 tile: no
                # mixed-engine tile, so no false WAW against ACT
                esi = self.espool.tile([128, 1024], I16, tag="esi", name="esi")
                if "nostrip" not in abl:
                    nc.vector.tensor_scalar(esi[:], s_psum[:], SCH_A, SCH_B,
                                            op0=mybir.AluOpType.mult,
                                            op1=mybir.AluOpType.add)
                es_tiles[(P, c)] = esi.bitcast(BF16)
                return
            es = self.espool.tile([128, 1024], BF16, tag="es", name="es")
            lo = 1024 - STRIP_X if STRIP_X else 1024
            if "noact" not in abl and lo:
                nc.scalar.activation(es[:, 0:lo], s_psum[:, 0:lo],
                                     EXPF, scale=0.25)
            if STRIP_X and "nostrip" not in abl:
                # NOTE: this write false-WAWs against the exp above (bitcast
                # defeats subtile ranges) — serializes ACT->DVE per stage.
                # Prefer DVE_EVERY whole-stage offload instead.
                nc.vector.tensor_scalar(es.bitcast(I16)[:, lo:1024],
                                        s_psum[:, lo:1024], SCH_A, SCH_B,
                                        op0=mybir.AluOpType.mult,
                                        op1=mybir.AluOpType.add)
            es_tiles[(P, c)] = es

        def av_stage(P, c):
            if "noav" in abl:
                es_tiles.pop((P, c))
                return
            if P == 0 and c == 0:
                hp_box["hp"] = self.hpsum.tile([128, R], F32, tag="heads",
                                               name="heads")
            hp = hp_box["hp"]
            es = es_tiles.pop((P, c))
            v_aug = ctx["v_aug"]
            g = 128 * c + 64 * P
            nc.tensor.matmul(hp[64 * P:64 * P + 32, :],
                             v_aug[:, g:g + 32], es[:, 0:512],
                             start=(c == 0), stop=(c == NCHUNK - 1),
                             tile_position=(0, 64 * P))
            nc.tensor.matmul(hp[64 * P + 32:64 * P + 64, :],
                             v_aug[:, g + 32:g + 64], es[:, 512:1024],
                             start=(c == 0), stop=(c == NCHUNK - 1),
                             tile_position=(0, 64 * P + 32))

        def epilogue():
            hp = hp_box.pop("hp")
            st = {}

            def ep_recip():
                recipb = self.sbops.tile([128, R], BF16, tag="recipb",
                                         name="recipb")
                with nc.allow_low_precision(
                        reason="recip feeds a bf16 matmul operand anyway"):
                    nc.vector.reciprocal(recipb[0:97, :], hp[0:97, :])
                st["recipb"] = recipb

            def ep_rb():
                ps = []
                for w in range(0, R, 256 if MPSUM256 else 512):
                    mw = 256 if MPSUM256 else 512
                    rb_psum = self.mpsum.tile([128, mw], F32, tag="m", name="m")
                    for h in range(H):
                        nc.tensor.matmul(rb_psum[32 * h:32 * h + 32, :],
                                         self.ones_rb[32 * h:32 * h + 1, :],
                                         st["recipb"][32 * h:32 * h + 1,
                                                      w:w + mw],
                                         start=True, stop=True,
                                         tile_position=(32 * h, 32 * h))
                    ps.append((w, mw, rb_psum))
                st["rb_psum"] = ps

            def ep_hpsb():
                hp_sb = self.sbops.tile([128, R], BF16, tag="hp_sb",
                                        name="hp_sb")
                with nc.allow_low_precision(
                        reason="hp feeds a bf16 matmul operand anyway"):
                    nc.vector.tensor_copy(hp_sb[:], hp[:])
                st["hp_sb"] = hp_sb

            def ep_rbcopy():
                rb_sb = self.sbops.tile([128, R], F32, tag="rb_sb", name="rb_sb")
                for w, mw, rb_psum in st["rb_psum"]:
                    nc.vector.tensor_copy(rb_sb[:, w:w + mw], rb_psum[:])
                st["rb"] = rb_sb

            def ep_mul():
                hn = self.consts.tile([128, R], BF16, tag="hn", name="hn")
                if EPI_HPSB:
                    for w, mw, rb_psum in st["rb_psum"]:
                        nc.vector.tensor_mul(hn[:, w:w + mw],
                                             st["hp_sb"][:, w:w + mw],
                                             rb_psum[:])
                else:
                    nc.vector.tensor_mul(hn[:], hp[:], st["rb"][:])
                ctx["hn"] = hn

            if EPI_HPSB:
                return [(60.0, ep_hpsb), (60.0, ep_recip), (260.0, ep_rb),
                        (60.0, ep_mul)]
            return [(60.0, ep_recip), (260.0, ep_rb), (60.0, ep_rbcopy),
                    (60.0, ep_mul)]

        o_dma = (nc.sync.dma_start if O_ON_SP else nc.scalar.dma_start)

        def o_chunk(c):
            def f():
                cs = slice(128 * c, 128 * (c + 1))
                hn, wo_sb = ctx["hn"], ctx["wo_sb"]
                o_psum = self.mpsum.tile([128, D], F32, tag="m", name="m")
                nc.tensor.matmul(o_psum[:], hn[:, cs], wo_sb[:],
                                 start=True, stop=True)
                o_sb = self.sbops.tile([128, D], F32, tag="o_sb", name="o_sb",
                                       bufs=2)
                nc.vector.tensor_copy(o_sb[:], o_psum[:])
                o_dma(out=self.out[cs, :], in_=o_sb[:])
            return f

        def o_pair(c):
            # chunks c, c+1 share one [128, 512] psum + ONE copy
            def f():
                hn, wo_sb = ctx["hn"], ctx["wo_sb"]
                o_psum = self.mpsum.tile([128, 2 * D], F32, tag="m", name="m")
                nc.tensor.matmul(o_psum[:, 0:D], hn[:, 128 * c:128 * (c + 1)],
                                 wo_sb[:], start=True, stop=True)
                nc.tensor.matmul(o_psum[:, D:2 * D],
                                 hn[:, 128 * (c + 1):128 * (c + 2)],
                                 wo_sb[:], start=True, stop=True)
                o_sb = self.sbops.tile([128, 2 * D], F32, tag="o_sb",
                                       name="o_sb", bufs=2)
                nc.vector.tensor_copy(o_sb[:], o_psum[:])
                o_dma(out=self.out[128 * c:128 * (c + 1), :],
                      in_=o_sb[:, 0:D])
                o_dma(out=self.out[128 * (c + 1):128 * (c + 2), :],
                      in_=o_sb[:, D:2 * D])
            return f

        ctx_next, prep = (self.prep_closures() if prep_next else (None, []))
        prep = deque(prep)

        stages = [(P, c) for P in range(2) for c in range(NCHUNK)]

        def retire(t):
            av_stage(*stages[t])
            if t == len(stages) - 1:
                if "noepi" in abl or "noav" in abl:
                    hp_box.clear()
                    return
                self.pri.extend(epilogue())
                if O_PAIR and not MPSUM256:
                    for c in range(0, R // 128, 2):
                        self.pri.append((500.0, o_pair(c)))
                else:
                    for c in range(R // 128):
                        self.pri.append((280.0, o_chunk(c)))

        for t, (P, c) in enumerate(stages):
            s_stage(P, c)
            if prep and t == PREP_STAGE:
                self.filler.extend(prep)
                prep.clear()
            self.pump(FILL_NS)
            if t >= AV_LAG:
                retire(t - AV_LAG)
        for t in range(len(stages) - AV_LAG, len(stages)):
            retire(t)
        if prep:
            self.filler.extend(prep)
        return ctx_next


_NC_CACHE = None


def _host_in_maps(query, key_value, W_q, W_k, W_v, W_o):
    q_bf = np.ascontiguousarray(query.astype(BF))
    kv_bf = np.ascontiguousarray(key_value.astype(BF))
    wqkv_h = np.zeros((D, 384), dtype=BF)
    wqt = np.transpose(W_q, (1, 0, 2))  # [D, H, K]
    wkt = np.transpose(W_k, (1, 0, 2))
    wvt = np.transpose(W_v, (1, 0, 2))
    for h in range(H):
        c0 = 32 * h
        wqkv_h[:, c0:c0 + K] = wqt[:, h, :].astype(BF)
        wqkv_h[:, 128 + c0:128 + c0 + K] = wkt[:, h, :].astype(BF)
        wqkv_h[:, 256 + c0 + 1:256 + c0 + 1 + K] = wvt[:, h, :].astype(BF)
    wqkv_hh = np.ascontiguousarray(np.concatenate(
        [wqkv_h[0:128].T, wqkv_h[128:256].T], axis=0))
    wo_h = np.zeros((128, D), dtype=BF)
    wo_r = W_o.reshape(H, K, D)
    for h in range(H):
        wo_h[32 * h + 1:32 * h + 1 + K, :] = wo_r[h].astype(BF)
    return [{"q": q_bf[c * R:(c + 1) * R], "kv": kv_bf, "wqkv": wqkv_hh,
             "wo": wo_h}
            for c in range(NCORES)]


def kernel(query, key_value, W_q, W_k, W_v, W_o):
    global _NC_CACHE, LAST_RESULTS
    if _NC_CACHE is None:
        _NC_CACHE = _build()
    nc = _NC_CACHE
    in_maps = _host_in_maps(query, key_value, W_q, W_k, W_v, W_o)
    res = run_bass_kernel_spmd(nc, in_maps, list(range(NCORES)), trace=TRACE)
    LAST_RESULTS = res
    return np.concatenate([res.results[c]["out"] for c in range(NCORES)], axis=0)
